# revision 2
# baseline (speedup 1.0000x reference)
"""Averaged-key circular-convolutional attention on 8 trn2 NeuronCores — v3.

vs baseline: fp8 DoubleRow conv (d-form + exact rank-1 mean term), DMA-engine
transposes (host undoes the hankel row-flip in gather()), row-tiled final
projection B-part, dual HWDGE queues + gpsimd DMA for outputs.

Math per (b,h): out_h = Circ(attn/N) @ V_h, attn = softmax(Q_h . mean(K_h) * SCALE).
  - K_avg = (sum_n x) @ Wk_h.T / N  (mean is linear)
  - attn = (1 + d)/N, d = N*attn - 1 (|d| <~ 0.1): Circ(attn/N) @ V =
    (colsum(V) + Circ(d) @ V) / N^2; colsum(V) = (sum_n x) @ Wv_h.T exactly;
    d-term via fp8 DoubleRow matmuls (d scaled by S=1000).
  - Circulant blocks = hankel windows of [dS; dS; dS[:256]] in DRAM (+1 strides
    only); the row flip is undone on the host (column reorder in gather()).
Sharding: 24 (b,h) -> 3 heads/core (core c: b=c//4, heads 3*(c%4)..+2);
host sums the 4 partials per b.
"""
import numpy as np
import ml_dtypes

N = 2048
C = 768
D = 64
SCALE = D ** -0.5
NB = 16
BLK = 128

S = 1000.0                # d scale for fp8
K1 = 1.0 / S              # poh -> ohf copy scale (dS-win @ V -> d-term)
EXSC = SCALE / N          # exp scale: z-psum (folded) = z * N

_CACHE = {}


def _build_nc(repeat=1, phases=5, dr=True, outq='gpsimd'):
    import concourse.bass as bass
    import concourse.tile as tile
    from concourse import bacc, mybir

    f32 = mybir.dt.float32
    f16 = mybir.dt.float16
    bf16 = mybir.dt.bfloat16
    f8 = mybir.dt.float8e4
    AX = mybir.AxisListType.X
    AXY = mybir.AxisListType.XY
    EXP = mybir.ActivationFunctionType.Exp
    IDN = mybir.ActivationFunctionType.Identity
    DR = mybir.MatmulPerfMode.DoubleRow
    ADD = mybir.AluOpType.add

    nc = bacc.Bacc("TRN2", target_bir_lowering=False, debug=False, num_devices=8)

    xtb = nc.dram_tensor("xtb", [128, 12288], bf16, kind="ExternalInput")
    wvb = nc.dram_tensor("wvb", [128, 1152], bf16, kind="ExternalInput")
    wqna = nc.dram_tensor("wqna", [128, 768], bf16, kind="ExternalInput")
    wqnb = nc.dram_tensor("wqnb", [64, 768], bf16, kind="ExternalInput")
    wkt = nc.dram_tensor("wkt", [128, 1152], bf16, kind="ExternalInput")
    wpa = nc.dram_tensor("wpa", [128, 768], bf16, kind="ExternalInput")
    wpbb = nc.dram_tensor("wpbb", [128, 768], bf16, kind="ExternalInput")
    bpv = nc.dram_tensor("bpv", [128, 6], f32, kind="ExternalInput")
    iden = nc.dram_tensor("iden", [128, 128], bf16, kind="ExternalInput")
    msum = nc.dram_tensor("msum", [48, 3], bf16, kind="ExternalInput")
    mbc = nc.dram_tensor("mbc", [3, 48], bf16, kind="ExternalInput")
    out = nc.dram_tensor("out", [C, N], f16, kind="ExternalOutput")
    f2a = nc.dram_tensor("f2a", [3, 4352], f8)

    with tile.TileContext(nc) as tc:
        with (
            tc.tile_pool(name="bigw", bufs=1) as bigw,
            tc.tile_pool(name="bigx", bufs=2) as bigx,
            tc.tile_pool(name="big", bufs=1) as big,
            tc.tile_pool(name="work", bufs=3) as work,
            tc.tile_pool(name="ps", bufs=3, space="PSUM") as ps,
            tc.tile_pool(name="pv", bufs=2, space="PSUM") as pv,
        ):
            # ---- static weights (loaded once per rep-stream; bufs=1 reuses) ----
            wqnA_sb = bigw.tile([BLK, C], bf16, tag="wqnA")
            nc.sync.dma_start(wqnA_sb[:, :], wqna[:, :])
            wqnB_sb = bigw.tile([D, C], bf16, tag="wqnB")
            nc.sync.dma_start(wqnB_sb[:, :], wqnb[:, :])
            wk_sb = bigw.tile([BLK, 1152], bf16, tag="wk")
            nc.sync.dma_start(wk_sb[:, :], wkt[:, :])
            wpA_sb = bigw.tile([BLK, C], bf16, tag="wpA")
            nc.sync.dma_start(wpA_sb[:, :], wpa[:, :])
            wpBB_sb = bigw.tile([BLK, C], bf16, tag="wpBB")
            nc.sync.dma_start(wpBB_sb[:, :], wpbb[:, :])
            bp_sb = bigw.tile([BLK, 6], f32, tag="bp")
            nc.sync.dma_start(bp_sb[:, :], bpv[:, :])
            iden_sb = bigw.tile([BLK, BLK], bf16, tag="iden")
            nc.sync.dma_start(iden_sb[:, :], iden[:, :])
            msum_sb = bigw.tile([48, 3], bf16, tag="msum")
            nc.sync.dma_start(msum_sb[:, :], msum[:, :])
            mbc_sb = bigw.tile([3, 48], bf16, tag="mbc")
            nc.sync.dma_start(mbc_sb[:, :], mbc[:, :])

            def emit_front():
                """load xt + wv/wz assembly for ONE rep; returns (xt_sb, wvz, xmb)."""
                wvz = big.tile([BLK, 6 * 195], bf16, tag="wvz")
                wvzv = wvz[:, :].rearrange("p (cc w) -> p cc w", w=195)
                nc.sync.dma_start(wvzv[:, :, 0:192],
                                  wvb[:, :].rearrange("p (cc w) -> p cc w", w=192))
                xt_sb = bigx.tile([BLK, 12288], bf16, tag="xt")
                for ch in range(8):
                    eng = nc.scalar if ch % 2 == 0 else nc.sync
                    eng.dma_start(xt_sb[:, ch * 1536:(ch + 1) * 1536],
                                  xtb[:, ch * 1536:(ch + 1) * 1536])
                xmq = work.tile([BLK, 48], f32, tag="xmq")
                for ch in range(8):
                    v = xt_sb[:, ch * 1536:(ch + 1) * 1536].rearrange(
                        "p (blk cc j) -> p cc blk j", blk=2, cc=6)
                    nc.vector.tensor_reduce(xmq[:, ch * 6:(ch + 1) * 6], v, axis=AXY,
                                            op=ADD)
                xm = work.tile([BLK, 6], f32, tag="xm")
                nc.vector.tensor_reduce(
                    xm[:, :], xmq[:, :].rearrange("p (ch cc) -> p cc ch", ch=8),
                    axis=AX, op=ADD)
                xmb = work.tile([BLK, 6], bf16, tag="xmb")
                nc.vector.tensor_copy(xmb[:, :], xm[:, :])
                return xt_sb, wvz, wvzv, xmb

            def emit_kwz(front):
                """kavg + wz matmuls (PE) -> wvz z-columns. Needs xmb."""
                xt_sb, wvz, wvzv, xmb = front
                kps = pv.tile([BLK, 1], f32, tag="pv", name="kpsA")
                for cc in range(6):
                    nc.tensor.matmul(kps[:, :], wk_sb[:, cc * 192:cc * 192 + BLK],
                                     xmb[:, cc:cc + 1], start=(cc == 0), stop=(cc == 5))
                kavgA = work.tile([BLK, 1], bf16, tag="kavgA")
                nc.vector.tensor_copy(kavgA[:, :], kps[:, :])
                kps2 = pv.tile([D, 1], f32, tag="pv", name="kpsB")
                for cc in range(6):
                    nc.tensor.matmul(kps2[:, :], wk_sb[:, cc * 192 + BLK:cc * 192 + 192],
                                     xmb[:, cc:cc + 1], start=(cc == 0), stop=(cc == 5))
                kavgB = work.tile([D, 1], bf16, tag="kavgB")
                nc.vector.tensor_copy(kavgB[:, :], kps2[:, :])
                for h in range(3):
                    if h == 0:
                        lwq, kav = wqnA_sb[0:D, :], kavgA[0:D, :]
                    elif h == 1:
                        lwq, kav = wqnA_sb[D:BLK, :], kavgA[D:BLK, :]
                    else:
                        lwq, kav = wqnB_sb[0:D, :], kavgB[0:D, :]
                    pwz = pv.tile([BLK, 6], f32, tag="pv", name=f"pwz{h}")
                    for cc in range(6):
                        nc.tensor.matmul(pwz[:, cc:cc + 1],
                                         lwq[:, cc * BLK:(cc + 1) * BLK],
                                         kav, start=True, stop=True)
                    nc.vector.tensor_copy(wvzv[:, :, 192 + h:193 + h],
                                          pwz[:, :].rearrange("p (cc o) -> p cc o", o=1))
                # vsum + bias for this rep
                vps = pv.tile([BLK, 1], f32, tag="pv", name="vpsA")
                for cc in range(6):
                    nc.tensor.matmul(vps[:, :], wvz[:, cc * 195:cc * 195 + BLK],
                                     xmb[:, cc:cc + 1], start=(cc == 0), stop=(cc == 5))
                vsA = work.tile([BLK, 1], bf16, tag="vsA")
                nc.vector.tensor_copy(vsA[:, :], vps[:, :])
                vps2 = pv.tile([D, 1], f32, tag="pv", name="vpsB")
                for cc in range(6):
                    nc.tensor.matmul(vps2[:, :], wvz[:, cc * 195 + BLK:cc * 195 + 192],
                                     xmb[:, cc:cc + 1], start=(cc == 0), stop=(cc == 5))
                vsB = work.tile([D, 1], bf16, tag="vsB")
                nc.vector.tensor_copy(vsB[:, :], vps2[:, :])
                bias2 = pv.tile([BLK, 6], f32, tag="pv", name="bias2")
                for cc in range(6):
                    nc.tensor.matmul(bias2[:, cc:cc + 1],
                                     wpA_sb[:, cc * BLK:(cc + 1) * BLK],
                                     vsA, start=True, stop=False, skip_group_check=True)
                    nc.tensor.matmul(bias2[:, cc:cc + 1],
                                     wpBB_sb[0:D, cc * BLK:(cc + 1) * BLK],
                                     vsB, start=False, stop=True, skip_group_check=True)
                biasC = work.tile([BLK, 6], f32, tag="biasC")
                nc.vector.tensor_tensor(biasC[:, :], bias2[:, :], bp_sb[:, :], op=ADD)
                return biasC

            def emit_main(front, biasC, nxt):
                """V-proj + softmax + conv + transposes + final for one rep.
                nxt: (next_front_emitter) called mid-conv, or None."""
                xt_sb, wvz, wvzv, xmb = front
                vt = big.tile([BLK, 3 * 31 * D], f8, tag="vt")
                vt4 = vt[:, :].rearrange("p (h blk dd) -> p h blk dd", h=3, dd=D)
                ztr2 = work.tile([BLK, 48], bf16, tag="ztr2")
                for blk in range(NB):
                    pvv = pv.tile([BLK, 195], f32, tag="pv", name=f"pvv{blk}")
                    for cc in range(6):
                        nc.tensor.matmul(
                            pvv[:, :],
                            xt_sb[:, (blk * 6 + cc) * BLK:(blk * 6 + cc + 1) * BLK],
                            wvz[:, cc * 195:(cc + 1) * 195],
                            start=(cc == 0), stop=(cc == 5))
                    vsrc = pvv[:, 0:192].rearrange("p (h dd) -> p h dd", h=3)
                    if blk % 4 == 3:
                        nc.vector.tensor_copy(vt4[:, :, blk, :], vsrc)
                    else:
                        nc.scalar.activation(vt4[:, :, blk, :], vsrc, IDN)
                    nc.vector.tensor_copy(ztr2[:, blk * 3:(blk + 1) * 3],
                                          pvv[:, 192:195])
                nc.scalar.activation(vt4[:, :, NB:31, :], vt4[:, :, 0:15, :], IDN)

                # blocked softmax -> dS
                zh = work.tile([BLK, 48], bf16, tag="zh")
                nc.vector.tensor_copy(
                    zh[:, :].rearrange("p (h blk) -> p h blk", h=3),
                    ztr2[:, :].rearrange("p (blk h) -> p h blk", h=3))
                zt2 = pv.tile([48, BLK], bf16, tag="pv", name="zt2")
                nc.tensor.transpose(zt2[:, :], zh[:, :], iden_sb[:, :])
                ez = work.tile([48, BLK], f32, tag="ez")
                esp = work.tile([48, 1], f32, tag="esp")
                nc.scalar.activation(ez[:, :], zt2[:, :], EXP, bias=0.0, scale=EXSC,
                                     accum_out=esp[:, :])
                espb = work.tile([48, 1], bf16, tag="espb")
                nc.vector.tensor_copy(espb[:, :], esp[:, :])
                es3 = pv.tile([3, 1], f32, tag="pv", name="es3")
                nc.tensor.matmul(es3[:, :], msum_sb[:, :], espb[:, :],
                                 start=True, stop=True)
                rin = work.tile([3, 1], f32, tag="rin")
                nc.vector.reciprocal(rin[:, :], es3[:, :])
                nc.scalar.mul(rin[:, :], rin[:, :], float(N) * S)
                rinb = work.tile([3, 1], bf16, tag="rinb")
                nc.vector.tensor_copy(rinb[:, :], rin[:, :])
                rb48 = pv.tile([48, 1], f32, tag="pv", name="rb48")
                nc.tensor.matmul(rb48[:, :], mbc_sb[:, :], rinb[:, :],
                                 start=True, stop=True)
                rb48s = work.tile([48, 1], f32, tag="rb48s")
                nc.vector.tensor_copy(rb48s[:, :], rb48[:, :])
                d2b = work.tile([48, BLK], f8, tag="d2b")
                nc.vector.tensor_scalar(d2b[:, :], ez[:, :], rb48s[:, :], -S,
                                        op0=mybir.AluOpType.mult, op1=ADD)
                for h in range(3):
                    e1, e2 = (nc.sync, nc.scalar) if h % 2 == 0 else (nc.scalar, nc.sync)
                    e1.dma_start(bass.AP(f2a, h * 4352, [[BLK, 16], [1, BLK]]),
                                 d2b[h * 16:(h + 1) * 16, :])
                    e2.dma_start(bass.AP(f2a, h * 4352 + 2048, [[BLK, 16], [1, BLK]]),
                                 d2b[h * 16:(h + 1) * 16, :])
                    e1.dma_start(bass.AP(f2a, h * 4352 + 4096, [[BLK, 2], [1, BLK]]),
                                 d2b[h * 16:h * 16 + 2, :])

                toep = [big.tile([BLK, 2176], f8, tag=f"toep{h}", name=f"toep{h}")
                        for h in range(3)]
                for h in range(3):
                    e1, e2 = (nc.sync, nc.scalar) if h % 2 == 0 else (nc.scalar, nc.sync)
                    e1.dma_start(toep[h][:, 0:512],
                                 bass.AP(f2a, h * 4352 + N - 127, [[1, BLK], [1, 512]]))
                    e2.dma_start(toep[h][:, 512:2176],
                                 bass.AP(f2a, h * 4352 + N - 127 + 512,
                                         [[1, BLK], [1, 1664]]))

                # conv (fp8 DoubleRow) + copies + transposes
                ohf01 = big.tile([BLK, N], bf16, tag="ohf01")
                o01v = ohf01[:, :].rearrange("p (b hh dd) -> p hh b dd", hh=2, dd=D)
                ohf2p = big.tile([BLK, 1024], bf16, tag="ohf2p")
                o2v = ohf2p[:, :].rearrange("p (b gg dd) -> p gg b dd", gg=2, dd=D)
                ohTA = big.tile([BLK, N], bf16, tag="ohTA")
                ohTB2 = big.tile([BLK, 1024], bf16, tag="ohTB2")
                for h in range(3):
                    poh = ps.tile([BLK, 1024], f32, tag="ps", name=f"poh{h}")
                    pview = poh[:, :].rearrange("p (ae par dd) -> p par ae dd",
                                                par=2, dd=D)
                    for s in range(8):
                        lhs = toep[h][:, 256 * s:256 * (s + 1)].rearrange(
                            "p (ko m) -> p ko m", ko=2)
                        for par in range(2):
                            base = (1984 * h) + (2 * s + par) * D
                            rhs = vt[:, base:base + 1024].rearrange(
                                "p (ae two dd) -> p two ae dd", two=2, dd=D)
                            nc.tensor.matmul(pview[:, par, :, :], lhs, rhs,
                                             start=(s == 0), stop=(s == 7),
                                             perf_mode=DR, skip_group_check=True)
                    srcv = poh[:, :].rearrange("p (b dd) -> p b dd", dd=D)
                    if h == 0:
                        nc.scalar.activation(o01v[:, 0, 0:8, :], srcv[:, 0:8, :],
                                             IDN, scale=K1)
                        nc.scalar.activation(o01v[:, 0, 8:16, :], srcv[:, 8:16, :],
                                             IDN, scale=K1)
                    elif h == 1:
                        nc.vector.tensor_scalar_mul(o01v[:, 1, 0:8, :],
                                                    srcv[:, 0:8, :], K1)
                        nc.vector.tensor_scalar_mul(o01v[:, 1, 8:16, :],
                                                    srcv[:, 8:16, :], K1)
                        nc.sync.dma_start_transpose(
                            ohTA[:, 0:1024].rearrange("p (b j) -> p b j", j=BLK),
                            ohf01[:, 0:1024])
                        nc.sync.dma_start_transpose(
                            ohTA[:, 1024:2048].rearrange("p (b j) -> p b j", j=BLK),
                            ohf01[:, 1024:2048])
                    else:
                        nc.scalar.activation(o2v[:, 0, :, :], srcv[:, 0:8, :],
                                             IDN, scale=K1)
                        nc.vector.tensor_scalar_mul(o2v[:, 1, :, :],
                                                    srcv[:, 8:16, :], K1)
                        nc.scalar.dma_start_transpose(
                            ohTB2[:, 0:512].rearrange("p (b j) -> p b j", j=BLK),
                            ohf2p[:, 0:512])
                        nc.scalar.dma_start_transpose(
                            ohTB2[:, 512:1024].rearrange("p (b j) -> p b j", j=BLK),
                            ohf2p[:, 512:1024])
                    if h == 1 and nxt is not None:
                        nxt()

                # final projection
                for cc in range(6):
                    pfs = []
                    for half in range(2):
                        pf = ps.tile([BLK, 1024], f32, tag="ps", name=f"pf{cc}_{half}")
                        pfs.append(pf)
                        for q in range(2):
                            nc.tensor.matmul(
                                pf[:, q * 512:(q + 1) * 512],
                                wpA_sb[:, cc * BLK:(cc + 1) * BLK],
                                ohTA[:, half * 1024 + q * 512:half * 1024 + (q + 1) * 512],
                                start=True, stop=False, skip_group_check=True)
                    for q in range(2):
                        nc.tensor.matmul(
                            pfs[0][:, q * 512:(q + 1) * 512],
                            wpBB_sb[0:D, cc * BLK:(cc + 1) * BLK],
                            ohTB2[0:D, q * 512:(q + 1) * 512],
                            start=False, stop=True, skip_group_check=True)
                        nc.tensor.matmul(
                            pfs[1][:, q * 512:(q + 1) * 512],
                            wpBB_sb[D:BLK, cc * BLK:(cc + 1) * BLK],
                            ohTB2[D:BLK, q * 512:(q + 1) * 512],
                            start=False, stop=True, skip_group_check=True)
                    fo = work.tile([BLK, N], f16, tag="fo")
                    nc.vector.tensor_scalar_add(fo[:, 0:1024], pfs[0][:, :],
                                                biasC[:, cc:cc + 1])
                    nc.scalar.activation(fo[:, 1024:2048], pfs[1][:, :], IDN,
                                         bias=biasC[:, cc:cc + 1], scale=1.0)
                    if outq == 'gpsimd':
                        nc.gpsimd.dma_start(out[cc * BLK:(cc + 1) * BLK, :], fo[:, :])
                    else:
                        eng = nc.sync if cc % 2 == 0 else nc.scalar
                        eng.dma_start(out[cc * BLK:(cc + 1) * BLK, :], fo[:, :])

            # ---- software-pipelined repeat loop ----
            front = emit_front()
            biasC = emit_kwz(front)
            state = {"front": front, "biasC": biasC}
            for rep in range(repeat):
                last = (rep == repeat - 1)
                if last:
                    nxt = None
                else:
                    def nxt():
                        f = emit_front()
                        state["next"] = (f, None)
                cur_front, cur_bias = state["front"], state["biasC"]
                emit_main(cur_front, cur_bias, nxt)
                if not last:
                    nf = state["next"][0]
                    nb = emit_kwz(nf)
                    state["front"], state["biasC"] = nf, nb
    nc.finalize()
    return nc


def _get_nc(repeat=1, phases=5, dr=True, outq='gpsimd'):
    key = ("nc", repeat, phases, dr, outq)
    if key not in _CACHE:
        _CACHE[key] = _build_nc(repeat, phases, dr, outq)
    return _CACHE[key]


def make_in_maps(x, Wq, Wk, Wv, Wp, bp):
    bf = ml_dtypes.bfloat16
    NN2 = float(N) * float(N)
    in_maps = []
    for core in range(8):
        b, g = core // 4, core % 4
        rows = slice(g * 192, (g + 1) * 192)
        xb = np.ascontiguousarray(x[b]).reshape(NB, BLK, 6, BLK)  # blk j cc c0
        xtb = np.ascontiguousarray(xb.transpose(3, 0, 2, 1)).reshape(BLK, 12288)
        wvg = Wv[rows].reshape(192, 6, BLK)  # o cc c0
        wvb = np.ascontiguousarray(wvg.transpose(2, 1, 0)).reshape(BLK, 1152)
        wkg = Wk[rows].reshape(192, 6, BLK)
        wkt = np.ascontiguousarray(wkg.transpose(2, 1, 0)).reshape(BLK, 1152)
        wpa = np.ascontiguousarray(Wp[:, g * 192:g * 192 + BLK].T) / NN2
        wpb = Wp[:, g * 192 + BLK:(g + 1) * 192].T / NN2  # [64, 768]
        wpbb = np.vstack([wpb, wpb])
        bpv = (bp if g == 0 else np.zeros_like(bp)).reshape(6, BLK).T
        iden = np.eye(BLK, dtype=np.float32)
        msum = np.zeros((48, 3), np.float32)
        for p in range(48):
            msum[p, p // 16] = 1.0
        in_maps.append({
            "iden": iden.astype(bf),
            "msum": msum.astype(bf),
            "mbc": np.ascontiguousarray(msum.T).astype(bf),
            "xtb": xtb.astype(bf),
            "wvb": wvb.astype(bf),
            "wqna": np.ascontiguousarray(Wq[rows][:BLK]).astype(bf),
            "wqnb": np.ascontiguousarray(Wq[rows][BLK:]).astype(bf),
            "wkt": wkt.astype(bf),
            "wpa": wpa.astype(bf),
            "wpbb": np.ascontiguousarray(wpbb).astype(bf),
            "bpv": np.ascontiguousarray(bpv).astype(np.float32),
        })
    return in_maps


def gather(results):
    outs = []
    for b in range(2):
        tot = results[4 * b]["out"].astype(np.float32)
        for g in range(1, 4):
            tot = tot + results[4 * b + g]["out"].astype(np.float32)
        # undo the within-block column flip from the hankel-window conv
        tot = tot.reshape(C, NB, BLK)[:, :, ::-1].reshape(C, N)
        outs.append(tot.T)
    return np.stack(outs, axis=0)


def run_spmd(in_maps, trace=False, **kw):
    from concourse.bass_utils import run_bass_kernel_spmd
    return run_bass_kernel_spmd(_get_nc(), in_maps, core_ids=list(range(8)),
                                trace=trace, **kw)


def kernel(x, Wq, Wk, Wv, Wp, bp):
    res = run_spmd(make_in_maps(np.asarray(x, np.float32), np.asarray(Wq, np.float32),
                                np.asarray(Wk, np.float32), np.asarray(Wv, np.float32),
                                np.asarray(Wp, np.float32), np.asarray(bp, np.float32)))
    return gather(res.results)


# revision 4
# speedup vs baseline: 1.1933x; 1.1933x over previous
"""Averaged-key circular-convolutional attention on 8 trn2 NeuronCores — v3.

vs baseline: fp8 DoubleRow conv (d-form + exact rank-1 mean term), DMA-engine
transposes (host undoes the hankel row-flip in gather()), row-tiled final
projection B-part, dual HWDGE queues + gpsimd DMA for outputs.

Math per (b,h): out_h = Circ(attn/N) @ V_h, attn = softmax(Q_h . mean(K_h) * SCALE).
  - K_avg = (sum_n x) @ Wk_h.T / N  (mean is linear)
  - attn = (1 + d)/N, d = N*attn - 1 (|d| <~ 0.1): Circ(attn/N) @ V =
    (colsum(V) + Circ(d) @ V) / N^2; colsum(V) = (sum_n x) @ Wv_h.T exactly;
    d-term via fp8 DoubleRow matmuls (d scaled by S=1000).
  - Circulant blocks = hankel windows of [dS; dS; dS[:256]] in DRAM (+1 strides
    only); the row flip is undone on the host (column reorder in gather()).
Sharding: 24 (b,h) -> 3 heads/core (core c: b=c//4, heads 3*(c%4)..+2);
host sums the 4 partials per b.
"""
import numpy as np
import ml_dtypes

N = 2048
C = 768
D = 64
SCALE = D ** -0.5
NB = 16
BLK = 128

S = 1000.0                # d scale for fp8
K1 = 1.0 / S              # poh -> ohf copy scale (dS-win @ V -> d-term)
EXSC = SCALE / N          # exp scale: z-psum (folded) = z * N

_CACHE = {}


def _build_nc(repeat=1, phases=5, dr=True, outq='gpsimd'):
    import concourse.bass as bass
    import concourse.tile as tile
    from concourse import bacc, mybir

    f32 = mybir.dt.float32
    f16 = mybir.dt.float16
    bf16 = mybir.dt.bfloat16
    f8 = mybir.dt.float8e4
    AX = mybir.AxisListType.X
    AXY = mybir.AxisListType.XY
    EXP = mybir.ActivationFunctionType.Exp
    IDN = mybir.ActivationFunctionType.Identity
    DR = mybir.MatmulPerfMode.DoubleRow
    ADD = mybir.AluOpType.add

    nc = bacc.Bacc("TRN2", target_bir_lowering=False, debug=False, num_devices=8)

    xtb = nc.dram_tensor("xtb", [128, 12288], bf16, kind="ExternalInput")
    wvb = nc.dram_tensor("wvb", [128, 1152], bf16, kind="ExternalInput")
    wqna = nc.dram_tensor("wqna", [128, 768], bf16, kind="ExternalInput")
    wqnb = nc.dram_tensor("wqnb", [64, 768], bf16, kind="ExternalInput")
    wkt = nc.dram_tensor("wkt", [128, 1152], bf16, kind="ExternalInput")
    wpa = nc.dram_tensor("wpa", [128, 768], bf16, kind="ExternalInput")
    wpbb = nc.dram_tensor("wpbb", [128, 768], bf16, kind="ExternalInput")
    bpv = nc.dram_tensor("bpv", [128, 6], f32, kind="ExternalInput")
    iden = nc.dram_tensor("iden", [128, 128], bf16, kind="ExternalInput")
    msum = nc.dram_tensor("msum", [48, 3], bf16, kind="ExternalInput")
    mbc = nc.dram_tensor("mbc", [3, 48], bf16, kind="ExternalInput")
    out = nc.dram_tensor("out", [C, N], f16, kind="ExternalOutput")
    f2a = nc.dram_tensor("f2a", [3, 4352], f8)

    with tile.TileContext(nc) as tc:
        with (
            tc.tile_pool(name="bigw", bufs=1) as bigw,
            tc.tile_pool(name="bigx", bufs=2) as bigx,
            tc.tile_pool(name="big", bufs=1) as big,
            tc.tile_pool(name="work", bufs=3) as work,
            tc.tile_pool(name="ps", bufs=3, space="PSUM") as ps,
            tc.tile_pool(name="pv", bufs=2, space="PSUM") as pv,
        ):
            # ---- static weights (loaded once per rep-stream; bufs=1 reuses) ----
            wqnA_sb = bigw.tile([BLK, C], bf16, tag="wqnA")
            nc.sync.dma_start(wqnA_sb[:, :], wqna[:, :])
            wqnB_sb = bigw.tile([D, C], bf16, tag="wqnB")
            nc.sync.dma_start(wqnB_sb[:, :], wqnb[:, :])
            wk_sb = bigw.tile([BLK, 1152], bf16, tag="wk")
            nc.sync.dma_start(wk_sb[:, :], wkt[:, :])
            wpA_sb = bigw.tile([BLK, C], bf16, tag="wpA")
            nc.sync.dma_start(wpA_sb[:, :], wpa[:, :])
            wpBB_sb = bigw.tile([BLK, C], bf16, tag="wpBB")
            nc.sync.dma_start(wpBB_sb[:, :], wpbb[:, :])
            bp_sb = bigw.tile([BLK, 6], f32, tag="bp")
            nc.sync.dma_start(bp_sb[:, :], bpv[:, :])
            iden_sb = bigw.tile([BLK, BLK], bf16, tag="iden")
            nc.sync.dma_start(iden_sb[:, :], iden[:, :])
            msum_sb = bigw.tile([48, 3], bf16, tag="msum")
            nc.sync.dma_start(msum_sb[:, :], msum[:, :])
            mbc_sb = bigw.tile([3, 48], bf16, tag="mbc")
            nc.sync.dma_start(mbc_sb[:, :], mbc[:, :])

            def emit_front():
                """load xt + wv/wz assembly for ONE rep; returns (xt_sb, wvz, xmb)."""
                wvz = big.tile([BLK, 6 * 195], bf16, tag="wvz")
                wvzv = wvz[:, :].rearrange("p (cc w) -> p cc w", w=195)
                nc.sync.dma_start(wvzv[:, :, 0:192],
                                  wvb[:, :].rearrange("p (cc w) -> p cc w", w=192))
                xt_sb = bigx.tile([BLK, 12288], bf16, tag="xt")
                for ch in range(8):
                    eng = nc.scalar if ch % 2 == 0 else nc.sync
                    eng.dma_start(xt_sb[:, ch * 1536:(ch + 1) * 1536],
                                  xtb[:, ch * 1536:(ch + 1) * 1536])
                xmq = work.tile([BLK, 48], f32, tag="xmq")
                for ch in range(8):
                    v = xt_sb[:, ch * 1536:(ch + 1) * 1536].rearrange(
                        "p (blk cc j) -> p cc blk j", blk=2, cc=6)
                    nc.vector.tensor_reduce(xmq[:, ch * 6:(ch + 1) * 6], v, axis=AXY,
                                            op=ADD)
                xm = work.tile([BLK, 6], f32, tag="xm")
                nc.vector.tensor_reduce(
                    xm[:, :], xmq[:, :].rearrange("p (ch cc) -> p cc ch", ch=8),
                    axis=AX, op=ADD)
                xmb = work.tile([BLK, 6], bf16, tag="xmb")
                nc.vector.tensor_copy(xmb[:, :], xm[:, :])
                return xt_sb, wvz, wvzv, xmb

            def emit_kwz(front):
                """kavg + wz matmuls (PE) -> wvz z-columns. Needs xmb."""
                xt_sb, wvz, wvzv, xmb = front
                kps = pv.tile([BLK, 1], f32, tag="pv", name="kpsA")
                for cc in range(6):
                    nc.tensor.matmul(kps[:, :], wk_sb[:, cc * 192:cc * 192 + BLK],
                                     xmb[:, cc:cc + 1], start=(cc == 0), stop=(cc == 5))
                kavgA = work.tile([BLK, 1], bf16, tag="kavgA")
                nc.vector.tensor_copy(kavgA[:, :], kps[:, :])
                kps2 = pv.tile([D, 1], f32, tag="pv", name="kpsB")
                for cc in range(6):
                    nc.tensor.matmul(kps2[:, :], wk_sb[:, cc * 192 + BLK:cc * 192 + 192],
                                     xmb[:, cc:cc + 1], start=(cc == 0), stop=(cc == 5))
                kavgB = work.tile([D, 1], bf16, tag="kavgB")
                nc.vector.tensor_copy(kavgB[:, :], kps2[:, :])
                for h in range(3):
                    if h == 0:
                        lwq, kav = wqnA_sb[0:D, :], kavgA[0:D, :]
                    elif h == 1:
                        lwq, kav = wqnA_sb[D:BLK, :], kavgA[D:BLK, :]
                    else:
                        lwq, kav = wqnB_sb[0:D, :], kavgB[0:D, :]
                    pwz = pv.tile([BLK, 6], f32, tag="pv", name=f"pwz{h}")
                    for cc in range(6):
                        nc.tensor.matmul(pwz[:, cc:cc + 1],
                                         lwq[:, cc * BLK:(cc + 1) * BLK],
                                         kav, start=True, stop=True)
                    nc.vector.tensor_copy(wvzv[:, :, 192 + h:193 + h],
                                          pwz[:, :].rearrange("p (cc o) -> p cc o", o=1))
                # vsum + bias for this rep
                vps = pv.tile([BLK, 1], f32, tag="pv", name="vpsA")
                for cc in range(6):
                    nc.tensor.matmul(vps[:, :], wvz[:, cc * 195:cc * 195 + BLK],
                                     xmb[:, cc:cc + 1], start=(cc == 0), stop=(cc == 5))
                vsA = work.tile([BLK, 1], bf16, tag="vsA")
                nc.vector.tensor_copy(vsA[:, :], vps[:, :])
                vps2 = pv.tile([D, 1], f32, tag="pv", name="vpsB")
                for cc in range(6):
                    nc.tensor.matmul(vps2[:, :], wvz[:, cc * 195 + BLK:cc * 195 + 192],
                                     xmb[:, cc:cc + 1], start=(cc == 0), stop=(cc == 5))
                vsB = work.tile([D, 1], bf16, tag="vsB")
                nc.vector.tensor_copy(vsB[:, :], vps2[:, :])
                bias2 = pv.tile([BLK, 6], f32, tag="pv", name="bias2")
                for cc in range(6):
                    nc.tensor.matmul(bias2[:, cc:cc + 1],
                                     wpA_sb[:, cc * BLK:(cc + 1) * BLK],
                                     vsA, start=True, stop=False, skip_group_check=True)
                    nc.tensor.matmul(bias2[:, cc:cc + 1],
                                     wpBB_sb[0:D, cc * BLK:(cc + 1) * BLK],
                                     vsB, start=False, stop=True, skip_group_check=True)
                biasC = work.tile([BLK, 6], f32, tag="biasC")
                nc.vector.tensor_tensor(biasC[:, :], bias2[:, :], bp_sb[:, :], op=ADD)
                return biasC

            def emit_main(front, biasC, nxt):
                """V-proj + softmax + conv + transposes + final for one rep.
                nxt: (next_front_emitter) called mid-conv, or None."""
                xt_sb, wvz, wvzv, xmb = front
                vt = big.tile([BLK, 3 * 31 * D], f8, tag="vt")
                vt4 = vt[:, :].rearrange("p (h blk dd) -> p h blk dd", h=3, dd=D)
                ztr2 = work.tile([BLK, 48], bf16, tag="ztr2")
                for blk in range(NB):
                    pvv = pv.tile([BLK, 195], f32, tag="pv", name=f"pvv{blk}")
                    for cc in range(6):
                        nc.tensor.matmul(
                            pvv[:, :],
                            xt_sb[:, (blk * 6 + cc) * BLK:(blk * 6 + cc + 1) * BLK],
                            wvz[:, cc * 195:(cc + 1) * 195],
                            start=(cc == 0), stop=(cc == 5))
                    vsrc = pvv[:, 0:192].rearrange("p (h dd) -> p h dd", h=3)
                    if blk % 4 == 3:
                        nc.vector.tensor_copy(vt4[:, :, blk, :], vsrc)
                    else:
                        nc.scalar.activation(vt4[:, :, blk, :], vsrc, IDN)
                    nc.vector.tensor_copy(ztr2[:, blk * 3:(blk + 1) * 3],
                                          pvv[:, 192:195])
                nc.scalar.activation(vt4[:, :, NB:24, :], vt4[:, :, 0:8, :], IDN)
                nc.scalar.activation(vt4[:, :, 24:31, :], vt4[:, :, 8:15, :], IDN)

                # blocked softmax -> dS
                zh = work.tile([BLK, 48], bf16, tag="zh")
                nc.vector.tensor_copy(
                    zh[:, :].rearrange("p (h blk) -> p h blk", h=3),
                    ztr2[:, :].rearrange("p (blk h) -> p h blk", h=3))
                zt2 = pv.tile([48, BLK], bf16, tag="pv", name="zt2")
                nc.tensor.transpose(zt2[:, :], zh[:, :], iden_sb[:, :])
                ez = work.tile([48, BLK], f32, tag="ez")
                esp = work.tile([48, 1], f32, tag="esp")
                nc.scalar.activation(ez[:, :], zt2[:, :], EXP, bias=0.0, scale=EXSC,
                                     accum_out=esp[:, :])
                espb = work.tile([48, 1], bf16, tag="espb")
                nc.vector.tensor_copy(espb[:, :], esp[:, :])
                es3 = pv.tile([3, 1], f32, tag="pv", name="es3")
                nc.tensor.matmul(es3[:, :], msum_sb[:, :], espb[:, :],
                                 start=True, stop=True)
                rin = work.tile([3, 1], f32, tag="rin")
                nc.vector.reciprocal(rin[:, :], es3[:, :])
                nc.scalar.mul(rin[:, :], rin[:, :], float(N) * S)
                rinb = work.tile([3, 1], bf16, tag="rinb")
                nc.vector.tensor_copy(rinb[:, :], rin[:, :])
                rb48 = pv.tile([48, 1], f32, tag="pv", name="rb48")
                nc.tensor.matmul(rb48[:, :], mbc_sb[:, :], rinb[:, :],
                                 start=True, stop=True)
                rb48s = work.tile([48, 1], f32, tag="rb48s")
                nc.vector.tensor_copy(rb48s[:, :], rb48[:, :])
                d2b = work.tile([48, BLK], f8, tag="d2b")
                nc.vector.tensor_scalar(d2b[:, :], ez[:, :], rb48s[:, :], -S,
                                        op0=mybir.AluOpType.mult, op1=ADD)
                for h in range(3):
                    e1, e2 = (nc.sync, nc.scalar) if h % 2 == 0 else (nc.scalar, nc.sync)
                    e1.dma_start(bass.AP(f2a, h * 4352, [[BLK, 16], [1, BLK]]),
                                 d2b[h * 16:(h + 1) * 16, :])
                    e2.dma_start(bass.AP(f2a, h * 4352 + 2048, [[BLK, 16], [1, BLK]]),
                                 d2b[h * 16:(h + 1) * 16, :])
                    e1.dma_start(bass.AP(f2a, h * 4352 + 4096, [[BLK, 2], [1, BLK]]),
                                 d2b[h * 16:h * 16 + 2, :])

                toep = [big.tile([BLK, 2176], f8, tag=f"toep{h}", name=f"toep{h}")
                        for h in range(3)]
                for h in range(3):
                    e1, e2 = (nc.sync, nc.scalar) if h % 2 == 0 else (nc.scalar, nc.sync)
                    e1.dma_start(toep[h][:, 0:256],
                                 bass.AP(f2a, h * 4352 + N - 127, [[1, BLK], [1, 256]]))
                    e2.dma_start(toep[h][:, 256:1216],
                                 bass.AP(f2a, h * 4352 + N - 127 + 256,
                                         [[1, BLK], [1, 960]]))
                    e1.dma_start(toep[h][:, 1216:2176],
                                 bass.AP(f2a, h * 4352 + N - 127 + 1216,
                                         [[1, BLK], [1, 960]]))

                # conv (fp8 DoubleRow) + copies + transposes
                ohf01 = big.tile([BLK, N], bf16, tag="ohf01")
                o01v = ohf01[:, :].rearrange("p (b hh dd) -> p hh b dd", hh=2, dd=D)
                ohf2p = big.tile([BLK, 1024], bf16, tag="ohf2p")
                o2v = ohf2p[:, :].rearrange("p (b gg dd) -> p gg b dd", gg=2, dd=D)
                ohTA = big.tile([BLK, N], bf16, tag="ohTA")
                ohTB2 = big.tile([BLK, 1024], bf16, tag="ohTB2")
                for h in range(3):
                    poh = ps.tile([BLK, 1024], f32, tag="ps", name=f"poh{h}")
                    pview = poh[:, :].rearrange("p (ae par dd) -> p par ae dd",
                                                par=2, dd=D)
                    for s in range(8):
                        lhs = toep[h][:, 256 * s:256 * (s + 1)].rearrange(
                            "p (ko m) -> p ko m", ko=2)
                        for par in range(2):
                            base = (1984 * h) + (2 * s + par) * D
                            rhs = vt[:, base:base + 1024].rearrange(
                                "p (ae two dd) -> p two ae dd", two=2, dd=D)
                            nc.tensor.matmul(pview[:, par, :, :], lhs, rhs,
                                             start=(s == 0), stop=(s == 7),
                                             perf_mode=DR, skip_group_check=True)
                    srcv = poh[:, :].rearrange("p (b dd) -> p b dd", dd=D)
                    if h == 0:
                        nc.scalar.activation(o01v[:, 0, 0:8, :], srcv[:, 0:8, :],
                                             IDN, scale=K1)
                        nc.scalar.activation(o01v[:, 0, 8:16, :], srcv[:, 8:16, :],
                                             IDN, scale=K1)
                    elif h == 1:
                        nc.vector.tensor_scalar_mul(o01v[:, 1, 0:8, :],
                                                    srcv[:, 0:8, :], K1)
                        nc.vector.tensor_scalar_mul(o01v[:, 1, 8:16, :],
                                                    srcv[:, 8:16, :], K1)
                        nc.sync.dma_start_transpose(
                            ohTA[:, 0:1024].rearrange("p (b j) -> p b j", j=BLK),
                            ohf01[:, 0:1024])
                        nc.sync.dma_start_transpose(
                            ohTA[:, 1024:2048].rearrange("p (b j) -> p b j", j=BLK),
                            ohf01[:, 1024:2048])
                    else:
                        nc.scalar.activation(o2v[:, 0, :, :], srcv[:, 0:8, :],
                                             IDN, scale=K1)
                        nc.vector.tensor_scalar_mul(o2v[:, 1, :, :],
                                                    srcv[:, 8:16, :], K1)
                        nc.scalar.dma_start_transpose(
                            ohTB2[:, 0:512].rearrange("p (b j) -> p b j", j=BLK),
                            ohf2p[:, 0:512])
                        nc.scalar.dma_start_transpose(
                            ohTB2[:, 512:1024].rearrange("p (b j) -> p b j", j=BLK),
                            ohf2p[:, 512:1024])
                    if h == 1 and nxt is not None:
                        nxt()

                # final projection
                for cc in range(6):
                    pfs = []
                    for half in range(2):
                        pf = ps.tile([BLK, 1024], f32, tag="ps", name=f"pf{cc}_{half}")
                        pfs.append(pf)
                        for q in range(2):
                            nc.tensor.matmul(
                                pf[:, q * 512:(q + 1) * 512],
                                wpA_sb[:, cc * BLK:(cc + 1) * BLK],
                                ohTA[:, half * 1024 + q * 512:half * 1024 + (q + 1) * 512],
                                start=True, stop=False, skip_group_check=True)
                    for q in range(2):
                        nc.tensor.matmul(
                            pfs[0][:, q * 512:(q + 1) * 512],
                            wpBB_sb[0:D, cc * BLK:(cc + 1) * BLK],
                            ohTB2[0:D, q * 512:(q + 1) * 512],
                            start=False, stop=True, skip_group_check=True)
                        nc.tensor.matmul(
                            pfs[1][:, q * 512:(q + 1) * 512],
                            wpBB_sb[D:BLK, cc * BLK:(cc + 1) * BLK],
                            ohTB2[D:BLK, q * 512:(q + 1) * 512],
                            start=False, stop=True, skip_group_check=True)
                    fo = work.tile([BLK, N], f16, tag="fo")
                    nc.vector.tensor_scalar_add(fo[:, 0:512], pfs[0][:, 0:512],
                                                biasC[:, cc:cc + 1])
                    nc.scalar.activation(fo[:, 512:1024], pfs[0][:, 512:1024], IDN,
                                         bias=biasC[:, cc:cc + 1], scale=1.0)
                    nc.vector.tensor_scalar_add(fo[:, 1024:1536], pfs[1][:, 0:512],
                                                biasC[:, cc:cc + 1])
                    nc.scalar.activation(fo[:, 1536:2048], pfs[1][:, 512:1024], IDN,
                                         bias=biasC[:, cc:cc + 1], scale=1.0)
                    if outq == 'gpsimd':
                        nc.gpsimd.dma_start(out[cc * BLK:(cc + 1) * BLK, :], fo[:, :])
                    else:
                        eng = nc.sync if cc % 2 == 0 else nc.scalar
                        eng.dma_start(out[cc * BLK:(cc + 1) * BLK, :], fo[:, :])

            # ---- software-pipelined repeat loop ----
            front = emit_front()
            biasC = emit_kwz(front)
            state = {"front": front, "biasC": biasC}
            for rep in range(repeat):
                last = (rep == repeat - 1)
                if last:
                    nxt = None
                else:
                    def nxt():
                        f = emit_front()
                        state["next"] = (f, None)
                cur_front, cur_bias = state["front"], state["biasC"]
                emit_main(cur_front, cur_bias, nxt)
                if not last:
                    nf = state["next"][0]
                    nb = emit_kwz(nf)
                    state["front"], state["biasC"] = nf, nb
    nc.finalize()
    return nc


def _get_nc(repeat=1, phases=5, dr=True, outq='gpsimd'):
    key = ("nc", repeat, phases, dr, outq)
    if key not in _CACHE:
        _CACHE[key] = _build_nc(repeat, phases, dr, outq)
    return _CACHE[key]


def make_in_maps(x, Wq, Wk, Wv, Wp, bp):
    bf = ml_dtypes.bfloat16
    NN2 = float(N) * float(N)
    in_maps = []
    for core in range(8):
        b, g = core // 4, core % 4
        rows = slice(g * 192, (g + 1) * 192)
        xb = np.ascontiguousarray(x[b]).reshape(NB, BLK, 6, BLK)  # blk j cc c0
        xtb = np.ascontiguousarray(xb.transpose(3, 0, 2, 1)).reshape(BLK, 12288)
        wvg = Wv[rows].reshape(192, 6, BLK)  # o cc c0
        wvb = np.ascontiguousarray(wvg.transpose(2, 1, 0)).reshape(BLK, 1152)
        wkg = Wk[rows].reshape(192, 6, BLK)
        wkt = np.ascontiguousarray(wkg.transpose(2, 1, 0)).reshape(BLK, 1152)
        wpa = np.ascontiguousarray(Wp[:, g * 192:g * 192 + BLK].T) / NN2
        wpb = Wp[:, g * 192 + BLK:(g + 1) * 192].T / NN2  # [64, 768]
        wpbb = np.vstack([wpb, wpb])
        bpv = (bp if g == 0 else np.zeros_like(bp)).reshape(6, BLK).T
        iden = np.eye(BLK, dtype=np.float32)
        msum = np.zeros((48, 3), np.float32)
        for p in range(48):
            msum[p, p // 16] = 1.0
        in_maps.append({
            "iden": iden.astype(bf),
            "msum": msum.astype(bf),
            "mbc": np.ascontiguousarray(msum.T).astype(bf),
            "xtb": xtb.astype(bf),
            "wvb": wvb.astype(bf),
            "wqna": np.ascontiguousarray(Wq[rows][:BLK]).astype(bf),
            "wqnb": np.ascontiguousarray(Wq[rows][BLK:]).astype(bf),
            "wkt": wkt.astype(bf),
            "wpa": wpa.astype(bf),
            "wpbb": np.ascontiguousarray(wpbb).astype(bf),
            "bpv": np.ascontiguousarray(bpv).astype(np.float32),
        })
    return in_maps


def gather(results):
    outs = []
    for b in range(2):
        tot = results[4 * b]["out"].astype(np.float32)
        for g in range(1, 4):
            tot = tot + results[4 * b + g]["out"].astype(np.float32)
        # undo the within-block column flip from the hankel-window conv
        tot = tot.reshape(C, NB, BLK)[:, :, ::-1].reshape(C, N)
        outs.append(tot.T)
    return np.stack(outs, axis=0)


def run_spmd(in_maps, trace=False, **kw):
    from concourse.bass_utils import run_bass_kernel_spmd
    return run_bass_kernel_spmd(_get_nc(), in_maps, core_ids=list(range(8)),
                                trace=trace, **kw)


def kernel(x, Wq, Wk, Wv, Wp, bp):
    res = run_spmd(make_in_maps(np.asarray(x, np.float32), np.asarray(Wq, np.float32),
                                np.asarray(Wk, np.float32), np.asarray(Wv, np.float32),
                                np.asarray(Wp, np.float32), np.asarray(bp, np.float32)))
    return gather(res.results)
